# revision 30
# baseline (speedup 1.0000x reference)
"""Trainium2 Bass kernel for nn_ConcatenationAggregator.

For each review r:
    out[r] = relu(concat(review_vecs[r],
                         user_vecs[adj_u[r]][perm_u],
                         item_vecs[adj_i[r]][perm_i]) @ W)

Strategy (pure data-parallel over reviews, 8 NeuronCores):
  - The feature permutations AND the W2/W3 blocks of W are folded into the
    tables on the host: UP = user_vecs @ W2p, IP = item_vecs @ W3p.  Then
    out[r] = relu(review_vecs[r] @ W1 + UP[adj_u[r]] + IP[adj_i[r]]), i.e.
    the gathered rows are pure elementwise addends and the only device
    matmul left is the review term.
  - Everything on device is bf16 except the f32 PSUM accumulator: review
    stream, projected tables, and the stored output.
  - Row gathers use the GPSIMD dma_gather ucode (int16 indices, <=1024
    indices per call — larger calls fault).  Tables are stored with 256B
    row stride (the ucode addresses rows in 256B quanta) but descriptors
    carry only the 128B live row: the stock wrapper insists on 256B
    multiples of elem_size, so `_raw_dma_gather` emits the instruction
    directly; hardware-validated bit-exact.  Halves gather DMA cost.
  - Reviews are globally sorted into 8 groups by (user-table 32K chunk,
    item-table 32K chunk) so rebased indices fit int16, and each group is
    split evenly across the 8 cores, so every core runs one shared program
    with minimal padding; the host un-permutes the output.
  - Layout: 1024-review chunks of 8 sub-tiles.  The review stream is
    host-transposed to feature-major [64, slots]; each sub-tile j is a
    [64, 128] stationary lhsT with W1 the [64, 64] moving rhs (the cheap
    orientation: 64 moving rows), producing a row-major [128 rows, 64]
    PSUM block per sub-tile (one PSUM bank per chunk).  Gathered rows
    arrive row-major, so the user+item contribution is one DVE add, a
    second DVE add accumulates into PSUM in place, relu runs on the
    otherwise-idle Activation engine, and the output is stored
    row-major-wrapped [128, slots*64/128] and unwrapped on the host.
    Output stores issue from the Activation HWDGE queue so SP never
    blocks ahead of the next chunk's loads; index tables travel as the
    unique 16 partitions and are replicated 8x on the DVE engine.
  - This toolchain build enforces ONE sync-wait slot per instruction, so
    tiny "observer" ops absorb extra cross-engine waits: a 1-column PE
    matmul takes the PSUM-recycle wait, small DVE/Act copies take the
    gather-completion and buffer-recycle waits, and the kernel-tail drain
    is split into single-wait drains.

Cost-model occupancy at 349us: Pool descriptor-gen 96% (994ns fixed +
0.34ns/desc per gather call, 2 calls per 1024 reviews — the structural
floor), DMA engines 78%, DVE 49%, Act 29%, PE 15%.
"""

import os
import types

import numpy as np
import ml_dtypes

import concourse.bacc as bacc
import concourse.mybir as mybir
import concourse.tile as tile
from concourse import ap_utils
from concourse.bass_utils import run_bass_kernel_spmd
from concourse.vector_clock import ScopedClock, VectorClock

F32 = mybir.dt.float32
BF16 = mybir.dt.bfloat16
I16 = mybir.dt.int16

NP_BF16 = ml_dtypes.bfloat16

N_CORES = 8
D = 64
DPAD = 128                 # padded table row (256B in bf16, dma_gather min)
SUB = 128                  # reviews per sub-tile
MAX_S = 8                  # sub-tiles per chunk (<=1024 gather indices)
TCH = 32768                # table chunk (int16 index range)

N_REVIEWS = 1_000_000
N_USERS = 100_000
N_ITEMS = 50_000
RPC = N_REVIEWS // N_CORES

BUFS = int(os.environ.get("KBUFS", "5"))
PREF = int(os.environ.get("KPREF", "4"))


def _split_drain_and_barrier(self, tick_clock, wait_clock):
    """Replacement for TileContext._drain_and_barrier: the stock tail drain
    waits on every live proc semaphore at once, which overflows this
    toolchain's one-sync-wait-per-instruction limit.  Emit one drain per
    semaphore instead."""
    gc = tick_clock.global_clock
    ticks = list(gc)
    idxs = [i for i, t in enumerate(ticks) if t > 0]
    for i in idxs:
        sub = [0] * len(ticks)
        sub[i] = ticks[i]
        drain_inst = self.nc.sync.drain()
        wait_clock.add_sem_waits(
            drain_inst.ins, ScopedClock({None: VectorClock(sub)}))
    if not idxs:
        drain_inst = self.nc.sync.drain()
        wait_clock.add_sem_waits(
            drain_inst.ins, ScopedClock({None: VectorClock(ticks)}))
    self.nc.all_engine_barrier()
    assert self.sems is not None
    popped = self.nc._tile_sem_poison_stack.pop()
    assert popped is self._sem_poison
    self.nc.clear_and_free_semaphores(list(self.sems.allocated().values()))
    self.nc.all_engine_barrier()


def _raw_dma_gather(gp, out_ap, in_ap, idxs_ap, num_idxs, num_idxs_reg,
                    elem_size, elem_step):
    """BassGpSimd.dma_gather without the elem_size_bytes%256 assert: the
    gather ucode strides in 256B quanta (stride_bytes_256) but transfers
    elem_size bytes per descriptor, so a 256B-stride table with 128B live
    rows gathers at half the descriptor cost.  Verified bit-exact on
    hardware (idx addresses the 256B-stride row, descriptors carry the
    first 128B)."""
    assert idxs_ap.dtype == mybir.dt.int16
    assert in_ap.dtype == out_ap.dtype
    assert ap_utils.ap_is_contiguous(in_ap.ap[1:])
    assert ap_utils.ap_is_contiguous(out_ap.ap[1:])
    assert ap_utils.ap_is_contiguous(idxs_ap.ap[1:])
    assert in_ap.ap[0][0] == elem_step
    assert num_idxs % 128 == 0
    assert out_ap.ap[-1][1] == elem_size
    stride_bytes = elem_step * mybir.dt.size(in_ap.dtype)
    stride_bytes_256 = stride_bytes // 256
    assert stride_bytes % 256 == 0 and 0 < stride_bytes_256 < 256
    _in_ap = gp.lower_ap_dma(in_ap, for_custom_bir_dma=True)
    _idxs_ap = gp.lower_ap(idxs_ap)
    _out_ap = gp.lower_ap(out_ap)
    return gp.add_instruction(
        mybir.InstDMAGatherAnt(
            name=gp.bass.get_next_instruction_name(),
            ins=[*_in_ap, _idxs_ap,
                 gp.lower_val_access(gp.to_reg(num_idxs_reg))],
            outs=[_out_ap],
            transpose=False,
            num_idxs=num_idxs,
            elem_size=elem_size,
            stride_bytes_256=stride_bytes_256,
            gen_mode=0,
            single_packet=True,
            queue_num=0,
            sbuf_tokens_per_rank=0,
            sbuf_free_dim_per_rank=0,
            sbuf_free_dim_pad_per_rank=0,
            sbuf_byte_offset=0,
        ))


def _chunk_list(s_per_group):
    """[(group, s_subtiles, row_base_slots, idxcol_base), ...] — shared by
    host packing and device program.  s values are <= MAX_S."""
    chunks = []
    row = 0
    col = 0
    for g, sg in enumerate(s_per_group):
        left = sg
        while left > 0:
            s = min(MAX_S, left)
            chunks.append((g, s, row, col))
            row += s * SUB
            col += s * 8
            left -= s
    return chunks


def _build_program(chunks, n_users, n_items):
    nc = bacc.Bacc("TRN2", target_bir_lowering=False, debug=False,
                   enable_asserts=False)
    padtot = sum(s for (_, s, _, _) in chunks) * SUB
    icols = padtot // 16

    rt_d = nc.dram_tensor("rt", [64, padtot], BF16, kind="ExternalInput")
    uidx_d = nc.dram_tensor("uidx", [16, icols], I16, kind="ExternalInput")
    iidx_d = nc.dram_tensor("iidx", [16, icols], I16, kind="ExternalInput")
    up_d = nc.dram_tensor("up", [n_users, DPAD], BF16, kind="ExternalInput")
    ip_d = nc.dram_tensor("ip", [n_items, DPAD], BF16, kind="ExternalInput")
    w1_d = nc.dram_tensor("w1", [64, 64], BF16, kind="ExternalInput")
    out_d = nc.dram_tensor("out2", [128, padtot // 2], BF16,
                           kind="ExternalOutput")

    T = len(chunks)
    RELU = mybir.ActivationFunctionType.Relu
    BYP = mybir.AluOpType.bypass
    ADD = mybir.AluOpType.add

    with tile.TileContext(nc) as tc:
        tc._drain_and_barrier = types.MethodType(_split_drain_and_barrier, tc)
        with tc.tile_pool(name="const", bufs=1) as constp, \
             tc.tile_pool(name="rtp", bufs=BUFS) as rtp, \
             tc.tile_pool(name="gup", bufs=BUFS) as gup, \
             tc.tile_pool(name="gip", bufs=BUFS) as gip, \
             tc.tile_pool(name="ttp", bufs=BUFS) as ttp, \
             tc.tile_pool(name="outp", bufs=BUFS) as outp, \
             tc.tile_pool(name="scr", bufs=1, space="PSUM") as scrp, \
             tc.tile_pool(name="mmp", bufs=BUFS, space="PSUM") as mmp:

            w1_t = constp.tile([64, 64], BF16)
            nc.sync.dma_start(out=w1_t[:], in_=w1_d.ap()[:, :])
            # Index tables: the gather ucode wants the 16-partition-wrapped
            # indices replicated across all 8 GPSIMD cores.  Only the unique
            # 16 partitions travel over DMA (per-group slices, so the first
            # gathers don't stall on the full upload); the 8x replication
            # runs on the lightly-loaded DVE engine.  Engine APs may only
            # start at partition 0/32/64/96, so DMA fills [0:16] and [16:32]
            # and DVE doubles [0:32]->[32:64] and [0:64]->[64:128].
            uidx_t = constp.tile([128, icols], I16)
            iidx_t = constp.tile([128, icols], I16)
            gcols: dict = {}
            for (g, s, _, col) in chunks:
                c0, c1 = gcols.get(g, (col, col))
                gcols[g] = (min(c0, col), max(c1, col + s * 8))
            dscr = constp.tile([16, 16], BF16)
            ascr = constp.tile([16, 16], BF16)

            def fill_idx(g):
                c0, c1 = gcols[g]
                for src_d, dst_t in ((uidx_d, uidx_t), (iidx_d, iidx_t)):
                    nc.sync.dma_start(out=dst_t[0:16, c0:c1],
                                      in_=src_d.ap()[:, c0:c1])
                    nc.scalar.dma_start(out=dst_t[16:32, c0:c1],
                                        in_=src_d.ap()[:, c0:c1])
                    # One HWDGE wait (max tick of the two loads) lands on the
                    # first copy; the second and the gathers ride the DVE
                    # clock.
                    nc.vector.tensor_copy(out=dst_t[32:64, c0:c1],
                                          in_=dst_t[0:32, c0:c1])
                    nc.vector.tensor_copy(out=dst_t[64:128, c0:c1],
                                          in_=dst_t[0:64, c0:c1])

            scratch = scrp.tile([128, 512], F32)
            # PE warmup: observe the weight-load (HWDGE) semaphore.
            nc.tensor.matmul(out=scratch[0:64, 0:64], lhsT=w1_t[:],
                             rhs=w1_t[:], start=True, stop=True)

            nreg = {}
            for (_, s, _, _) in chunks:
                if s not in nreg:
                    nreg[s] = nc.gpsimd.to_reg(s * SUB)

            ubase = [g // 2 * TCH for g in range(8)]
            usize = [min(TCH, n_users - b) for b in ubase]
            ibase = [g % 2 * TCH for g in range(8)]
            isize = [min(TCH, n_items - b) for b in ibase]

            rt_tiles = [None] * T
            gu_tiles = [None] * T
            gi_tiles = [None] * T
            ps_tiles = [None] * T
            o_tiles = [None] * T

            last_g = [-1]

            def issue_loads(t):
                g, s, row, col = chunks[t]
                if g != last_g[0]:
                    fill_idx(g)
                    last_g[0] = g
                rt_t = rtp.tile([64, MAX_S * 128], BF16, tag="rt")
                nc.sync.dma_start(
                    out=rt_t[:, :s * 128],
                    in_=rt_d.ap()[:, row: row + s * 128])
                gu_t = gup.tile([128, MAX_S * 64], BF16, tag="gu")
                _raw_dma_gather(
                    nc.gpsimd,
                    out_ap=gu_t[:, :s * 64].rearrange(
                        "p (n d) -> p n d", d=64),
                    in_ap=up_d.ap()[ubase[g]:ubase[g] + usize[g], :],
                    idxs_ap=uidx_t[:, col:col + s * 8],
                    num_idxs=s * SUB, num_idxs_reg=nreg[s],
                    elem_size=64, elem_step=DPAD)
                gi_t = gip.tile([128, MAX_S * 64], BF16, tag="gi")
                _raw_dma_gather(
                    nc.gpsimd,
                    out_ap=gi_t[:, :s * 64].rearrange(
                        "p (n d) -> p n d", d=64),
                    in_ap=ip_d.ap()[ibase[g]:ibase[g] + isize[g], :],
                    idxs_ap=iidx_t[:, col:col + s * 8],
                    num_idxs=s * SUB, num_idxs_reg=nreg[s],
                    elem_size=64, elem_step=DPAD)
                rt_tiles[t], gu_tiles[t], gi_tiles[t] = rt_t, gu_t, gi_t

            def issue_matmuls(t):
                _, s, _, _ = chunks[t]
                rt_t = rt_tiles[t]
                ps = mmp.tile([128, MAX_S * 64], F32, tag="mm")
                # 1-column dummy absorbs the PSUM-recycle wait so the first
                # real matmul carries only the rt-load wait.
                nc.tensor.matmul(out=ps[0:64, 0:1], lhsT=w1_t[:],
                                 rhs=w1_t[:, 0:1], start=True, stop=True)
                for j in range(s):
                    nc.tensor.matmul(
                        out=ps[:, j * 64:(j + 1) * 64],
                        lhsT=rt_t[:, j * 128:(j + 1) * 128],
                        rhs=w1_t[:], start=True, stop=True)
                ps_tiles[t] = ps

            def issue_elemwise(t):
                _, s, _, _ = chunks[t]
                n = s * 64
                gu_t, gi_t = gu_tiles[t], gi_tiles[t]
                ps = ps_tiles[t]
                # Observer: absorb the gather-completion wait so the first
                # add keeps a single wait slot.
                nc.vector.tensor_copy(out=dscr[:, :], in_=gu_t[0:16, 0:16])
                t_t = ttp.tile([128, MAX_S * 64], BF16, tag="t")
                nc.vector.scalar_tensor_tensor(
                    out=t_t[:, :n], in0=gu_t[:, :n], scalar=0.0,
                    in1=gi_t[:, :n], op0=BYP, op1=ADD)
                # psum += t  (in-place on PSUM; waits only on the last matmul)
                nc.vector.scalar_tensor_tensor(
                    out=ps[:, :n], in0=ps[:, :n], scalar=0.0,
                    in1=t_t[:, :n], op0=BYP, op1=ADD)

            def issue_relu(t):
                _, s, row, _ = chunks[t]
                n = s * 64
                o_t = outp.tile([128, MAX_S * 64], BF16, tag="o")
                # Observer: absorb the o_t store-recycle wait.
                nc.scalar.activation(out=ascr[:, :], in_=ascr[:, :],
                                     func=RELU)
                nc.scalar.activation(out=o_t[:, :n], in_=ps_tiles[t][:, :n],
                                     func=RELU)
                o_tiles[t] = o_t

            def issue_store(t):
                # Issued from the Activation HWDGE queue: the wait on the
                # relu is same-engine there, so SP's sequencer never blocks
                # ahead of the next chunk's loads.
                _, s, row, _ = chunks[t]
                nc.scalar.dma_start(
                    out=out_d.ap()[:, row // 2: row // 2 + s * 64],
                    in_=o_tiles[t][:, :s * 64])

            # Software-pipelined emission (see module docstring).
            for tt in range(min(PREF, T)):
                issue_loads(tt)
            for t in range(T):
                if t + PREF < T:
                    issue_loads(t + PREF)
                issue_matmuls(t)
                issue_elemwise(t)
                issue_relu(t)
                issue_store(t)
    nc.finalize()
    return nc


_PROGRAM_CACHE: dict = {}


def _get_program(chunk_key, n_users, n_items):
    key = (chunk_key, n_users, n_items)
    if key not in _PROGRAM_CACHE:
        _PROGRAM_CACHE[key] = (
            _build_program(_chunk_list(list(chunk_key)), n_users, n_items))
    return _PROGRAM_CACHE[key]


def _wrap_idx(flat_sorted, chunks):
    """Rebased int16 indices [PADTOT] -> [16, PADTOT//16] in dma_gather's
    wrapped layout: per chunk block [16, 8*s] with block[p, m] =
    flat[m*16 + p]; the device replicates across the 8 16-partition
    groups."""
    cols = []
    for (_, s, row, _) in chunks:
        blk = flat_sorted[row:row + s * SUB].reshape(s * 8, 16).T  # [16, 8s]
        cols.append(blk)
    return np.ascontiguousarray(np.concatenate(cols, axis=1))


def _run(review_vecs, user_vecs, item_vecs, W,
         review_user_adj, review_item_adj, perm_u, perm_i,
         n_cores, rpc):
    n_users = user_vecs.shape[0]
    n_items = item_vecs.shape[0]

    W = np.asarray(W, np.float32)
    W1 = np.ascontiguousarray(W[0:64])
    W2 = W[64:128]
    W3 = W[128:192]
    perm_u = np.asarray(perm_u, np.int64)
    perm_i = np.asarray(perm_i, np.int64)
    W2p = np.empty_like(W2)
    W2p[perm_u] = W2
    W3p = np.empty_like(W3)
    W3p[perm_i] = W3

    user_vecs = np.asarray(user_vecs, np.float32)
    item_vecs = np.asarray(item_vecs, np.float32)
    # Fold W2p/W3p into the tables; pad rows to 128 elems (256B in bf16).
    UP = np.zeros((n_users, DPAD), NP_BF16)
    UP[:, :64] = (user_vecs @ W2p).astype(NP_BF16)
    IP = np.zeros((n_items, DPAD), NP_BF16)
    IP[:, :64] = (item_vecs @ W3p).astype(NP_BF16)
    W1b = np.ascontiguousarray(W1.astype(NP_BF16))

    review_vecs = np.asarray(review_vecs, np.float32)
    au_all = np.asarray(review_user_adj, np.int64)
    ai_all = np.asarray(review_item_adj, np.int64)

    # Group ALL reviews by (user 32K chunk, item 32K chunk) and split each
    # group's reviews evenly across the cores: every core runs the same
    # chunk structure with minimal padding.
    grp_all = (au_all // TCH) * 2 + (ai_all // TCH)
    order_all = np.argsort(grp_all, kind="stable")
    counts_all = np.bincount(grp_all, minlength=8)
    gstart = np.concatenate([[0], np.cumsum(counts_all)])
    s_per_group = []
    for ctot in counts_all:
        per_core_max = -(-int(ctot) // n_cores)
        s_per_group.append(-(-per_core_max // SUB))
    chunk_key = tuple(s_per_group)
    chunks = _chunk_list(s_per_group)
    padtot = sum(s for (_, s, _, _) in chunks) * SUB

    nc = _get_program(chunk_key, n_users, n_items)

    in_maps = []
    slotmaps = []
    for c in range(n_cores):
        slotmap = np.full(padtot, -1, np.int64)
        row = 0
        for g in range(8):
            tot = int(counts_all[g])
            base, rem = divmod(tot, n_cores)
            cnt = base + (1 if c < rem else 0)
            off = c * base + min(c, rem)
            ids = order_all[gstart[g] + off: gstart[g] + off + cnt]
            slotmap[row:row + cnt] = ids
            row += s_per_group[g] * SUB
        valid = slotmap >= 0
        sl = np.where(valid, slotmap, 0)

        rv_sorted = np.where(valid[:, None], review_vecs[sl], 0.0)
        slot_g = np.repeat(np.arange(8), np.array(s_per_group) * SUB)
        u_reb = np.where(valid, au_all[sl] - (slot_g // 2) * TCH,
                         0).astype(np.int16)
        i_reb = np.where(valid, ai_all[sl] - (slot_g % 2) * TCH,
                         0).astype(np.int16)

        in_maps.append({
            "rt": np.ascontiguousarray(rv_sorted.T.astype(NP_BF16)),
            "uidx": _wrap_idx(u_reb, chunks),
            "iidx": _wrap_idx(i_reb, chunks),
            "up": UP,
            "ip": IP,
            "w1": W1b,
        })
        slotmaps.append((slotmap, valid))

    res = run_bass_kernel_spmd(nc, in_maps, core_ids=list(range(n_cores)))

    out = np.empty((n_cores * rpc, 64), np.float32)
    for c in range(n_cores):
        o2 = np.asarray(res.results[c]["out2"])
        # [128, padtot//2] wrapped row-major -> [padtot, 64]
        out_sorted = np.ascontiguousarray(
            o2.reshape(128, padtot // 128, 64).transpose(1, 0, 2)
        ).reshape(padtot, 64).astype(np.float32)
        slotmap, valid = slotmaps[c]
        out[slotmap[valid]] = out_sorted[valid]
    return out


def kernel(**inputs) -> np.ndarray:
    return _run(
        inputs["review_vecs"], inputs["user_vecs"], inputs["item_vecs"],
        inputs["W"], inputs["review_user_adj"], inputs["review_item_adj"],
        inputs["perm_u"], inputs["perm_i"],
        n_cores=N_CORES, rpc=RPC)
